# revision 1
# baseline (speedup 1.0000x reference)
"""Trainium2 Bass kernel for the (misordered-scale) MultiHeadAttention problem.

Problem (per batch b of 8, one NeuronCore each):
  qk = x @ Wqk.T + bqk            # [2048, 512], channel c = 2*(h*64+e) + {0:q, 1:k}
  v  = x @ Wv.T  + bv             # [2048, 256], channel c = h*64+e
  S_h = q_h @ k_h.T               # [2048, 2048] per head (e=64)
  attn = softmax(S, -1) / 16
  out_h = attn_h @ v_h            # [2048, 64]
  out = concat_h(out_h) @ Wo.T + bo   # [2048, 1024]

Strategy: data-parallel over batch across 8 cores (no collectives).  All
matmuls run at the PE's full 1 cycle/row rate with fp32 PSUM accumulation:
q/k/S and the projections in fp16 (2^-11 rounding), the exp(S)->AV path in
bf16 (softmax spans ~78 e-folds; fp16 cannot hold it -- max logit ~51).
x and the weights are cast fp32->fp16 with SWDGE cast-DMAs (HBM->HBM,
pipelined in 512-token blocks) and transposed into feature-major SBUF
layout with the hardware DMA-transpose xbar (PE/DVE never touch the
transpose).  Attention computes S^T tiles [128 j, 512 i], with the two
heads of a pair emitted adjacently at array row offsets 0/64 so they
execute concurrently on the PE; exp runs on ACT straight out of PSUM
(fused bias -8 for headroom); the AV matmul's stationary operand
[v_h | ones] makes PSUM row 64 the softmax denominator for free.
Normalization broadcasts 1/denom along partitions via a rank-1 PE matmul
(bf16 hi/lo split, fp32 accumulate).  The out-projection uses y^T as the
stationary operand so the output is produced token-major and stores are
contiguous.  Biases are applied via precomputed broadcast tiles during
PSUM evacuation on the DVE.
"""

import numpy as np
from contextlib import ExitStack

import concourse.bass as bass
import concourse.mybir as mybir
import concourse.tile as tile
from concourse import bacc
from concourse import bass_utils

FP32 = mybir.dt.float32
BF16 = mybir.dt.bfloat16
FP16 = mybir.dt.float16
AF = mybir.ActivationFunctionType
ALU = mybir.AluOpType

B = 8
N = 2048          # tokens per batch
D = 1024          # model dim
H = 4             # heads
E = 64            # per-head dim after the einops split
HD = 256          # H*E (v channels / Wo contraction dim)
NCORES = 8

DC = D // 128      # 8 d-chunks of 128
NIB = N // 512     # 4 i-blocks of 512
NJB = N // 128     # 16 j-blocks of 128
INV_SCALE = 1.0 / 16.0  # 1/sqrt(HEAD_DIM=256)
# exp(S - 8): headroom offset for the exp path (max logit ~51 -> e^43 fits
# bf16 comfortably); the offset cancels exactly in the softmax normalization.
EXP_BIAS = -8.0


def _build_kernel(nc: bass.Bass, tc: tile.TileContext, out_ap, x, wqk, bqk, wv, bv, wo, bo,
                  reps: int = 1, debug_outs=None):
    ctx = ExitStack()
    with ctx:
        consts = ctx.enter_context(tc.tile_pool(name="consts", bufs=1))
        dram = ctx.enter_context(tc.tile_pool(name="dram", bufs=1, space="DRAM"))
        exps_pool = ctx.enter_context(tc.tile_pool(name="exps", bufs=5))
        osb_pool = ctx.enter_context(tc.tile_pool(name="osb", bufs=4))
        norm_pool = ctx.enter_context(tc.tile_pool(name="norm", bufs=3))
        s_pool = ctx.enter_context(tc.tile_pool(name="spsum", bufs=2, space="PSUM"))
        av_pool = ctx.enter_context(tc.tile_pool(name="avpsum", bufs=1, space="PSUM"))
        misc_pool = ctx.enter_context(tc.tile_pool(name="miscpsum", bufs=2, space="PSUM"))

        # ---------------- persistent SBUF tensors ----------------
        xt = consts.tile([128, DC, N], FP16)          # x^T: xt[p, dc, t] = x[t, dc*128+p]
        wqT = consts.tile([128, DC, HD], FP16)        # Wq^T: [d, c]
        wkT = consts.tile([128, DC, HD], FP16)
        wvT = consts.tile([128, DC, HD], FP16)
        woT = consts.tile([128, 2, D], FP16)          # Wo^T: [c, do]
        qT = consts.tile([128, 2, N], FP16)           # q^T: [c, i] ; c-chunk cc, partition p -> c=cc*128+p
        kT = consts.tile([128, 2, N], FP16)
        yT = consts.tile([128, 2, N], FP16)           # concat-head attn out, feature-major
        vh = consts.tile([128, NJB, H, 66], BF16)     # [j, jb, h, 0:64]=v_h, [...,64]=1.0
        bq_sb = consts.tile([128, 2, 1], FP32)        # q bias per partition (c)
        bk_sb = consts.tile([128, 2, 1], FP32)
        bv_row = consts.tile([1, HD], FP32)           # v bias as K=1 matmul rhs
        bo_row = consts.tile([1, D], FP32)
        ones32 = consts.tile([1, 128], FP32)
        ones = consts.tile([1, 512], FP16)
        onesbf = consts.tile([1, 512], BF16)
        ones_col = consts.tile([128, 1], FP16)

        expb = consts.tile([128, 1], FP32)
        vbc = consts.tile([128, HD], FP32)       # bv broadcast over tokens
        obc = consts.tile([128, 2, 512], FP32)   # bo broadcast over tokens
        nc.vector.memset(ones[:], 1.0)
        nc.vector.memset(ones32[:], 1.0)
        nc.vector.memset(onesbf[:], 1.0)
        nc.vector.memset(ones_col[:], 1.0)
        nc.vector.memset(expb[:], EXP_BIAS)
        nc.vector.memset(vh[:, :, :, 64:66], 1.0)

        # ---------------- DRAM staging (bf16 casts) ----------------
        x_bf = dram.tile([N, D], FP16)
        wq_bf = dram.tile([HD, D], FP16)
        wk_bf = dram.tile([HD, D], FP16)
        wv_bf = dram.tile([HD, D], FP16)
        wo_bf = dram.tile([D, HD], FP16)

        # De-interleave Wqk rows: q rows are 2c, k rows are 2c+1.
        wqk_r = wqk.rearrange("(c s) d -> s c d", s=2)
        bqk_r = bqk.rearrange("(c s) -> s c", s=2)

        # The d (contraction) axis uses a permuted internal layout: SBUF chunk
        # dc at partition p holds global d = p*8 + dc.  This is consistent
        # between xt and all W^T tiles (d is purely internal), and lets one
        # full-width transpose-DMA fill all 8 chunks via a 3D output AP.
        # x is cast+transposed in 512-token blocks so the projections pipeline
        # with x's arrival (each token block carries all of d).
        # bias loads ride the ACT HWDGE ring so they don't delay the SP ring's
        # weight/x transposes.
        nc.scalar.dma_start(bv_row[0:1, :], bv[:])
        nc.scalar.dma_start(bo_row[0:1, :], bo[:])
        for cb in range(2):
            nc.scalar.dma_start(bq_sb[:, cb, :], bqk_r[0, cb * 128:(cb + 1) * 128])
            nc.scalar.dma_start(bk_sb[:, cb, :], bqk_r[1, cb * 128:(cb + 1) * 128])
        nc.gpsimd.dma_start(wq_bf[:], wqk_r[0])
        nc.sync.dma_start(wqT[:], wq_bf[:], transpose=True)
        nc.gpsimd.dma_start(x_bf[0:512, :], x[0:512, :])
        nc.sync.dma_start(xt[:, :, 0:512], x_bf[0:512, :], transpose=True)
        nc.gpsimd.dma_start(wk_bf[:], wqk_r[1])
        nc.sync.dma_start(wkT[:], wk_bf[:], transpose=True)
        nc.gpsimd.dma_start(x_bf[512:1024, :], x[512:1024, :])
        nc.sync.dma_start(xt[:, :, 512:1024], x_bf[512:1024, :], transpose=True)
        # wv is only needed once the (lagged) v projection starts, wo only for
        # the first out-projection -- keep them behind the early x chunks.
        nc.gpsimd.dma_start(wv_bf[:], wv[:])
        nc.sync.dma_start(wvT[:], wv_bf[:], transpose=True)
        for tb in range(2, 4):
            ts_ = slice(tb * 512, (tb + 1) * 512)
            nc.gpsimd.dma_start(x_bf[ts_, :], x[ts_, :])
            nc.sync.dma_start(xt[:, :, ts_], x_bf[ts_, :], transpose=True)
        for g in range(2):
            # Wo^T must match yT's c-layout (c = cc*128 + p): per-chunk.
            cs = slice(g * 128, (g + 1) * 128)
            nc.gpsimd.dma_start(wo_bf[:, cs], wo[:, cs])
            nc.sync.dma_start(woT[:, g, :], wo_bf[:, cs], transpose=True)

        # biases broadcast along tokens (partition axis) via rank-1 PE matmuls,
        # so per-tile K=1 bias matmuls and their PE streams are not needed.
        bb = misc_pool.tile([128, 512], FP32, tag="mm")
        nc.tensor.matmul(bb[:, 0:HD], lhsT=ones32[:], rhs=bv_row[:],
                         start=True, stop=True)
        nc.vector.tensor_copy(vbc[:], bb[:, 0:HD])
        for ob in range(2):
            bb2 = misc_pool.tile([128, 512], FP32, tag="mm")
            nc.tensor.matmul(bb2[:], lhsT=ones32[:],
                             rhs=bo_row[:, ob * 512:(ob + 1) * 512],
                             start=True, stop=True)
            nc.vector.tensor_copy(obc[:, ob, :], bb2[:])

        # ---------------- phase 1: q/k/v projections, per token block ----------
        # qk: psum[c_loc, i] = sum_d W*T[d, c] * xT[d, i]   (feature-major)
        # v:  psum[j_loc, c] = sum_d xT[d, j] * WvT[d, c]   (token-major)
        # Token block outer so projections pipeline with x's arrival.
        # (reps>1 repeats the compute phases for differential timing.)
        for _rep in range(reps):
            _compute_phases(nc, tc, out_ap, misc_pool, s_pool, av_pool, exps_pool,
                            osb_pool, norm_pool, xt, wqT, wkT, wvT, woT, qT, kT, yT,
                            vh, bq_sb, bk_sb, bv_row, bo_row, ones, onesbf, ones_col, expb,
                            vbc, obc)
        if debug_outs:
            locs = dict(xt=xt, wqT=wqT, wkT=wkT, wvT=wvT, woT=woT,
                        qT=qT, kT=kT, yT=yT, vh=vh)
            for name, dst in debug_outs.items():
                nc.gpsimd.dma_start(dst, locs[name][:])


def _compute_phases(nc, tc, out_ap, misc_pool, s_pool, av_pool, exps_pool, osb_pool,
                    norm_pool, xt, wqT, wkT, wvT, woT, qT, kT, yT, vh,
                    bq_sb, bk_sb, bv_row, bo_row, ones, onesbf, ones_col, expb, vbc, obc):
    def v_proj_block(ib):
        for jb in range(ib * 4, (ib + 1) * 4):
            pv = misc_pool.tile([128, 512], FP32, tag="mm")
            for dc in range(DC):
                nc.tensor.matmul(
                    pv[:, 0:HD],
                    lhsT=xt[:, dc, jb * 128:(jb + 1) * 128],
                    rhs=wvT[:, dc, :],
                    start=(dc == 0),
                    stop=(dc == DC - 1),
                )
            # single strided evacuation + bias: psum [128, (h e)] -> vh[:, jb, h, e]
            nc.vector.tensor_tensor(vh[:, jb, :, 0:64], pv[:, 0:HD], vbc[:], ALU.add)

    if True:
        for ib in range(NIB):
            for cb in range(2):
                for wT, b_sb, dstT in ((wqT, bq_sb, qT), (wkT, bk_sb, kT)):
                    pp = misc_pool.tile([128, 512], FP32, tag="mm")
                    for dc in range(DC):
                        nc.tensor.matmul(
                            pp[:],
                            lhsT=wT[:, dc, cb * 128:(cb + 1) * 128],
                            rhs=xt[:, dc, ib * 512:(ib + 1) * 512],
                            start=(dc == 0),
                            stop=(dc == DC - 1),
                        )
                    # evacuate + bias (per-partition) + cast to fp16 on DVE
                    # (keeps the ACT queue free for the attention exps)
                    nc.vector.tensor_scalar(
                        dstT[:, cb, ib * 512:(ib + 1) * 512], pp[:],
                        b_sb[:, cb, :], None, ALU.add,
                    )
            # v projection lags one token block: wvT arrives after wq/wk, and
            # this keeps PE fed with qk work meanwhile.
            if ib > 0:
                v_proj_block(ib - 1)
        v_proj_block(NIB - 1)

        # ---------------- phase 2+3: attention, out-proj interleaved ----------
        # S^T tile [j, i] = sum_e kT_h[e, j] * qT_h[e, i]; exp on ACT; AV pair
        # col-tiled into one [128, 512] psum (h_even rows 0:64, h_odd 64:128);
        # denominators via packed M=1 ones-matmuls into dn rows {0, 32}.
        # Adjacent S (row-tiled 0/64) and AV (col-tiled 0/64) pairs execute
        # concurrently on the PE array.
        def oproj_tile(it, ob):
            # out[i, do] = sum_c yT[c, i] * WoT[c, do] + bo[do]
            tsl = slice(it * 128, (it + 1) * 128)
            po = misc_pool.tile([128, 512], FP32, tag="mm")
            osl = slice(ob * 512, (ob + 1) * 512)
            for cc2 in range(2):
                nc.tensor.matmul(
                    po[:],
                    lhsT=yT[:, cc2, tsl],
                    rhs=woT[:, cc2, osl],
                    start=(cc2 == 0), stop=(cc2 == 1),
                )
            osb = osb_pool.tile([128, 512], FP32)
            nc.vector.tensor_tensor(osb[:], po[:], obc[:, ob, :], ALU.add)
            nc.sync.dma_start(out_ap[tsl, osl], osb[:])

        from collections import deque
        pending = deque()

        for ib in range(NIB):
            isl = slice(ib * 512, (ib + 1) * 512)
            for cc in range(2):          # head pair (2*cc, 2*cc+1)
                av0 = av_pool.tile([65, 512], FP32, tag="av0")
                av1 = av_pool.tile([65, 512], FP32, tag="av1")
                for jb in range(NJB):
                    jsl = slice(jb * 128, (jb + 1) * 128)
                    sp = s_pool.tile([128, 1024], FP32)
                    nc.tensor.matmul(
                        sp[:, 0:512],
                        lhsT=kT[0:64, cc, jsl], rhs=qT[0:64, cc, isl],
                        start=True, stop=True,
                    )
                    nc.tensor.matmul(
                        sp[:, 512:1024],
                        lhsT=kT[64:128, cc, jsl], rhs=qT[64:128, cc, isl],
                        start=True, stop=True,
                    )
                    ex = exps_pool.tile([128, 1024], BF16)
                    nc.scalar.activation(ex[:], sp[:], AF.Exp, bias=expb[:])
                    first, last = (jb == 0), (jb == NJB - 1)
                    nc.tensor.matmul(
                        av0[:],
                        lhsT=vh[:, jb, 2 * cc, 0:65], rhs=ex[:, 0:512],
                        start=first, stop=last,
                    )
                    nc.tensor.matmul(
                        av1[:],
                        lhsT=vh[:, jb, 2 * cc + 1, 0:65], rhs=ex[:, 512:1024],
                        start=first, stop=last,
                    )
                    if jb % 4 == 3 and pending:
                        oproj_tile(*pending.popleft())
                # evacuate av quickly (frees PSUM), then normalize from SBUF:
                # y^T_h = out^T_h * (1/(16*denom)), broadcast along partitions
                # via a rank-1 PE matmul.
                for hh, av in ((0, av0), (1, av1)):
                    pb = hh * 64
                    avs = norm_pool.tile([65, 512], FP32, tag=f"avs{hh}")
                    nc.vector.tensor_copy(avs[:], av[:])
                    # r = 1/denom in fp32.  r spans ~1e-22..1e0 (denominators
                    # up to ~1e13 from huge diagonal logits), so the 16-bit
                    # broadcast operand must be bf16 (fp16 underflows).  A
                    # bf16 hi/lo split accumulated in PSUM keeps ~2^-16
                    # precision on the normalization.
                    rec32 = norm_pool.tile([1, 512], FP32, tag=f"rec32{hh}")
                    nc.vector.reciprocal(rec32[:], avs[64:65, :])
                    rhi = norm_pool.tile([1, 512], BF16, tag=f"rhi{hh}")
                    nc.vector.tensor_copy(rhi[:], rec32[:])
                    rlo = norm_pool.tile([1, 512], BF16, tag=f"rlo{hh}")
                    with nc.allow_low_precision(reason="lo part of bf16 hi/lo split"):
                        nc.vector.tensor_tensor(rlo[:], rec32[:], rhi[:], ALU.subtract)
                    bc = misc_pool.tile([128, 512], FP32, tag="mm")
                    nc.tensor.matmul(bc[pb:pb + 64, :], lhsT=onesbf[:, 0:64],
                                     rhs=rhi[:], start=True, stop=False)
                    nc.tensor.matmul(bc[pb:pb + 64, :], lhsT=onesbf[:, 0:64],
                                     rhs=rlo[:], start=False, stop=True)
                    bcs = norm_pool.tile([64, 512], FP32, tag=f"bcs{hh}")
                    nc.vector.tensor_scalar(bcs[:], bc[pb:pb + 64, :], INV_SCALE, None, ALU.mult)
                    nc.vector.tensor_tensor(
                        yT[pb:pb + 64, cc, isl], avs[0:64, :], bcs[:], ALU.mult,
                    )
            for sub in range(4):
                pending.append((ib * 4 + sub, 0))
                pending.append((ib * 4 + sub, 1))
        while pending:
            oproj_tile(*pending.popleft())


_CACHE: dict = {}


def _get_compiled(reps: int = 1):
    key = ("nc", reps)
    if key in _CACHE:
        return _CACHE[key]
    nc = bacc.Bacc(
        "TRN2", target_bir_lowering=False, debug=False, num_devices=NCORES,
    )
    x = nc.dram_tensor("x", (N, D), FP32, kind="ExternalInput").ap()
    wqk = nc.dram_tensor("Wqk", (2 * HD, D), FP32, kind="ExternalInput").ap()
    bqk = nc.dram_tensor("bqk", (2 * HD,), FP32, kind="ExternalInput").ap()
    wv = nc.dram_tensor("Wv", (HD, D), FP32, kind="ExternalInput").ap()
    bv = nc.dram_tensor("bv", (HD,), FP32, kind="ExternalInput").ap()
    wo = nc.dram_tensor("Wo", (D, HD), FP32, kind="ExternalInput").ap()
    bo = nc.dram_tensor("bo", (D,), FP32, kind="ExternalInput").ap()
    out = nc.dram_tensor("out", (N, D), FP32, kind="ExternalOutput").ap()

    with tile.TileContext(nc) as tc:
        _build_kernel(nc, tc, out, x, wqk, bqk, wv, bv, wo, bo, reps=reps)
    nc.compile()
    _CACHE[key] = nc
    return nc


def run_cores(in_maps, trace=False, **kw):
    nc = _get_compiled()
    return bass_utils.run_bass_kernel_spmd(
        nc, in_maps, core_ids=list(range(NCORES)), trace=trace, **kw
    )


def kernel(x, Wqk, bqk, Wv, bv, Wo, bo):
    x = np.asarray(x, dtype=np.float32)
    in_maps = [
        {
            "x": np.ascontiguousarray(x[c]),
            "Wqk": np.asarray(Wqk, np.float32),
            "bqk": np.asarray(bqk, np.float32),
            "Wv": np.asarray(Wv, np.float32),
            "bv": np.asarray(bv, np.float32),
            "Wo": np.asarray(Wo, np.float32),
            "bo": np.asarray(bo, np.float32),
        }
        for c in range(NCORES)
    ]
    # The axon tunnel occasionally returns a glitched execution (transient
    # non-finite garbage); retry a couple of times in that case.
    for _attempt in range(3):
        res = run_cores(in_maps)
        out = np.stack([res.results[c]["out"] for c in range(NCORES)], axis=0)
        if np.isfinite(out).all():
            break
    return out

